# revision 44
# baseline (speedup 1.0000x reference)
"""DRAW forward kernel for Trainium2 (8 NeuronCores, pure data parallelism).

Hardcoded for: BATCH=256, image 128x128, grid N=32, H=512, Z=128, T=16.

Design notes:
  - Each core gets 32 batch rows + a replica of global row 255 (local row
    32). The reference broadcasts sigmoid(reconsts[-1]) (= row 255) to all
    rows; row 255's trajectory is self-contained, so every core tracks it
    locally -> zero collectives.
  - Everything (weights + state) is SBUF-resident across the 16 steps.
  - Dense GEMMs: layout B (weights stationary as bf16 lhsT tiles,
    activations feature-major [feat, 33] moving, fp32 PSUM accumulate).
  - Attention never materializes x_hat: F_y@(X-S)@F_x^T = x_attn -
    F_y@S@F_x^T where S = sigmoid(recon_replica) is shared across rows
    (one bulk matmul computes S@F_x^T for all rows).
  - Filterbanks via a single fused ScalarE pass per tile:
    Derivative_Erf(x) = 2/sqrt(pi)*exp(-x^2); the 2/sqrt(pi) folds into
    the row-sum normalization exactly (eps scaled by the same factor).
  - Per-(row, grid) scale/bias scalars are produced by one small
    constant-selector matmul (no gathers/transposes needed).
"""

import sys

for _p in ("/opt/trn_rl_repo",):
    if _p not in sys.path:
        sys.path.insert(0, _p)

import contextlib

import numpy as np

import concourse.bass as bass  # noqa: F401
import concourse.bacc as bacc
import concourse.mybir as mybir
from concourse import tile
from concourse.bass_utils import run_bass_kernel_spmd

F32 = mybir.dt.float32
BF16 = mybir.dt.bfloat16
AF = mybir.ActivationFunctionType
ALU = mybir.AluOpType
NPBF = mybir.dt.np(BF16)

A_ = 128
B_ = 128
NG = 32
H = 512
Z = 128
T = 16
PATCH = NG * NG
BATCH = 256
NCORES = 8
BL = 33            # local rows: 32 + replica
NT = 9             # ceil(33*32/128) (i,n)-tiles per filter bank

LN_SQRT_HALF = float(np.log(np.sqrt(0.5)))
LN_DELTA_SCALE = float(np.log((max(A_, B_) - 1) / (NG - 1)))
EPS_ADJ = 1e-8


# ----------------------------------------------------------------------------
# host-side layout prep
# ----------------------------------------------------------------------------

def _wT_tiles(w, npdt):
    """W [M, K] -> lhsT tiles [128, n_kt*n_mt*128], free order (kt, mt, m)."""
    M, K = w.shape
    n_mt, n_kt = M // 128, K // 128
    wt = w.T.reshape(n_kt, 128, n_mt, 128).transpose(1, 0, 2, 3)
    return np.ascontiguousarray(wt.reshape(128, -1).astype(npdt))


def _perm_r_feature(k, p):
    """r-part K-tile k, partition p -> feature h*1024 + n*32 + m."""
    h, q = k // 8, k % 8
    m, n4 = p // 4, p % 4
    return h * 1024 + (q * 4 + n4) * 32 + m


def _prep_w_ih_enc(w):
    n_mt = 12
    out = np.zeros((128, 20 * n_mt * 128), np.float32)
    wt = w.T  # [2560, 1536]
    for k in range(20):
        if k < 16:
            feats = np.array([_perm_r_feature(k, p) for p in range(128)])
        else:
            feats = 2048 + 128 * (k - 16) + np.arange(128)
        blk = wt[feats]
        for mt in range(n_mt):
            out[:, (k * n_mt + mt) * 128:(k * n_mt + mt + 1) * 128] = \
                blk[:, mt * 128:(mt + 1) * 128]
    return np.ascontiguousarray(out.astype(NPBF))


def _prep_w_wr(w, b):
    """Rows permuted: M-tile u, partition r = n*4 + s%4 <- feature n*32+s,
    s in [4u, 4u+4)."""
    perm = np.zeros(1024, np.int64)
    for u in range(8):
        for r in range(128):
            n, s4 = r // 4, r % 4
            perm[u * 128 + r] = n * 32 + (4 * u + s4)
    tiles = _wT_tiles(w[perm], NPBF)
    bias = np.ascontiguousarray(b[perm].reshape(8, 128).T.astype(np.float32))
    return tiles, bias


def _sel_consts():
    grid = np.arange(NG, dtype=np.float32) - 0.5 * (NG - 1)
    A = np.zeros((36, 128), np.float32)
    Ag = np.zeros((36, 128), np.float32)
    for i in range(36):
        for p in range(128):
            if i % 4 == p // 32:
                A[i, p] = 1.0
                Ag[i, p] = grid[p % 32]
    sela = np.zeros((100, 128), np.float32)
    sela[0:36] = A
    sela[64:100] = Ag
    selb = np.zeros((36, 9), np.float32)
    for i in range(36):
        selb[i, i // 4] = 1.0
    return sela, selb


def prepare_inputs(inputs):
    f32 = np.float32
    x = np.asarray(inputs["x"], f32)
    eps = np.asarray(inputs["eps"], f32)

    sh = {}
    sh["w_ihT_enc"] = _prep_w_ih_enc(np.asarray(inputs["W_ih_enc"], f32))
    sh["w_hhT_enc"] = _wT_tiles(np.asarray(inputs["W_hh_enc"], f32), NPBF)
    sh["w_ihT_dec"] = _wT_tiles(np.asarray(inputs["W_ih_dec"], f32), NPBF)
    sh["w_hhT_dec"] = _wT_tiles(np.asarray(inputs["W_hh_dec"], f32), NPBF)
    wmuls = np.concatenate([np.asarray(inputs["W_mu"], f32),
                            np.asarray(inputs["W_ls"], f32)], 0)
    sh["w_mulsT"] = _wT_tiles(wmuls, f32)
    sh["w_wrT"], sh["bias_wr"] = _prep_w_wr(
        np.asarray(inputs["W_wr"], f32), np.asarray(inputs["b_wr"], f32))

    def _pad5(w):
        wp = np.zeros((8, 512), f32)
        wp[:5] = w
        return np.ascontiguousarray(
            wp.T.reshape(4, 128, 8).transpose(1, 0, 2).reshape(128, 32))

    sh["w_raT"] = _pad5(np.asarray(inputs["W_ra"], f32))
    sh["w_waT"] = _pad5(np.asarray(inputs["W_wa"], f32))
    for nm, key in (("bias_ra", "b_ra"), ("bias_wa", "b_wa")):
        b = np.zeros((8, 1), f32)
        b[:5, 0] = np.asarray(inputs[key], f32)
        sh[nm] = b

    def _cols(v):
        return np.ascontiguousarray(v.reshape(-1, 128).T.astype(f32))

    bie = np.asarray(inputs["b_ih_enc"], f32)
    bhe = np.asarray(inputs["b_hh_enc"], f32)
    bid = np.asarray(inputs["b_ih_dec"], f32)
    bhd = np.asarray(inputs["b_hh_dec"], f32)
    sh["bias_enc_rz"] = _cols(bie[:1024] + bhe[:1024])
    sh["bias_enc_in"] = _cols(bie[1024:])
    sh["bias_enc_hn"] = _cols(bhe[1024:])
    sh["bias_dec_rz"] = _cols(bid[:1024] + bhd[:1024])
    sh["bias_dec_in"] = _cols(bid[1024:])
    sh["bias_dec_hn"] = _cols(bhd[1024:])
    sh["bias_muls"] = _cols(np.concatenate([np.asarray(inputs["b_mu"], f32),
                                            np.asarray(inputs["b_ls"], f32)]))
    sh["pconst"] = np.ascontiguousarray(
        np.tile(np.arange(128, dtype=f32)[None, :], (128, 1)))
    sela, selb = _sel_consts()
    sh["sela"] = np.ascontiguousarray(sela)
    sh["selb"] = np.ascontiguousarray(selb)
    sh["ident"] = np.ascontiguousarray(np.eye(128, dtype=f32).astype(NPBF))
    sh["identf"] = np.ascontiguousarray(np.eye(8, dtype=f32))

    per_core = []
    xf = x.reshape(BATCH, A_, B_)
    for c in range(NCORES):
        rows = list(range(32 * c, 32 * c + 32)) + [BATCH - 1]
        xT = xf[rows].transpose(2, 0, 1).reshape(128, BL * A_)
        eT = eps[:, rows, :].transpose(2, 0, 1).reshape(128, T * BL)
        m = dict(sh)
        m["xT"] = np.ascontiguousarray(xT.astype(NPBF))
        m["epsT"] = np.ascontiguousarray(eT.astype(f32))
        per_core.append(m)
    return per_core


DRAM_SPECS = [
    ("w_ihT_enc", [128, 20 * 12 * 128], BF16),
    ("w_hhT_enc", [128, 4 * 12 * 128], BF16),
    ("w_ihT_dec", [128, 1 * 12 * 128], BF16),
    ("w_hhT_dec", [128, 4 * 12 * 128], BF16),
    ("w_mulsT", [128, 4 * 2 * 128], F32),
    ("w_wrT", [128, 4 * 8 * 128], BF16),
    ("w_raT", [128, 32], F32),
    ("w_waT", [128, 32], F32),
    ("bias_ra", [8, 1], F32),
    ("bias_wa", [8, 1], F32),
    ("bias_enc_rz", [128, 8], F32),
    ("bias_enc_in", [128, 4], F32),
    ("bias_enc_hn", [128, 4], F32),
    ("bias_dec_rz", [128, 8], F32),
    ("bias_dec_in", [128, 4], F32),
    ("bias_dec_hn", [128, 4], F32),
    ("bias_muls", [128, 2], F32),
    ("bias_wr", [128, 8], F32),
    ("pconst", [128, 128], F32),
    ("sela", [100, 128], F32),
    ("selb", [36, 9], F32),
    ("ident", [128, 128], BF16),
    ("identf", [8, 8], F32),
    ("xT", [128, BL * A_], BF16),
    ("epsT", [128, T * BL], F32),
]


# ----------------------------------------------------------------------------
# program builder
# ----------------------------------------------------------------------------

def build_program(t_steps=T, debug_names=(), history=False):
    nc = bacc.Bacc("TRN2", target_bir_lowering=False, debug=False,
                   num_devices=NCORES)
    dram = {n: nc.dram_tensor(n, s, d, kind="ExternalInput").ap()
            for n, s, d in DRAM_SPECS}
    out_d = nc.dram_tensor("out_sig", [128, BL * B_], F32,
                           kind="ExternalOutput").ap()
    with tile.TileContext(nc) as tc:
        _build(tc, nc, dram, out_d, t_steps, debug_names, history)
    nc.compile()
    return nc


def _build(tc, nc, dram, out_d, t_steps, debug_names=(), history=False):
    sb = {}

    def sbuf(name, shape, dt):
        ap = nc.alloc_sbuf_tensor("sb_" + name, list(shape), dt).ap()
        sb[name] = ap
        return ap

    for name, shape, dt in DRAM_SPECS:
        sbuf(name, shape, dt)

    hdec = sbuf("hdec", [128, 4 * BL], F32)
    henc = sbuf("henc", [128, 4 * BL], F32)
    hdec_bf = sbuf("hdec_bf", [128, 4 * BL], BF16)
    henc_bf = sbuf("henc_bf", [128, 4 * BL], BF16)
    recon = sbuf("recon", [128, BL * B_], F32)
    zT_bf = sbuf("zT_bf", [128, BL], BF16)
    rT_bf = sbuf("rT_bf", [128, 16 * BL], BF16)
    wT_bf = sbuf("wT_bf", [128, 8 * BL], BF16)
    wblk = sbuf("wblk", [128, NG * BL], BF16)       # 4 replicated bases
    rbuf = sbuf("rbuf", [32, 2 * NG * BL], BF16)
    st_bf = sbuf("st_bf", [128, 128], BF16)
    sd_bf = sbuf("sd_bf", [128, 128], BF16)
    zsb = sbuf("zsb", [128, NG * BL], BF16)
    szneg = sbuf("szneg", [128, NT * 128], BF16)
    g_x = sbuf("g_x", [128, NT * 128], F32)
    g_y = sbuf("g_y", [128, NT * 128], F32)
    fx_n = sbuf("fx_n", [128, NT * 128], BF16)
    fy_n = sbuf("fy_n", [128, NT * 128], BF16)
    fxw_n = sbuf("fxw_n", [128, NT * 128], BF16)
    fyw_n = sbuf("fyw_n", [128, NT * 128], BF16)
    fxT = sbuf("fxT", [128, NT * 128], BF16)
    fyT = sbuf("fyT", [128, NT * 128], BF16)
    sums = sbuf("sums", [128, 4 * NT], F32)
    rcps = sbuf("rcps", [128, 4 * NT], F32)
    scal_r = sbuf("scal_r", [128, 5 * NT], F32)
    scal_w = sbuf("scal_w", [128, 5 * NT], F32)
    bx_r = sbuf("bx_r", [128, NT], F32)
    by_r = sbuf("by_r", [128, NT], F32)
    bx_w = sbuf("bx_w", [128, NT], F32)
    by_w = sbuf("by_w", [128, NT], F32)
    ball = sbuf("ball", [100, 5 * NT], F32)
    p5r = sbuf("p5r", [8, BL], F32)
    p5c = sbuf("p5c", [36, 8], F32)
    der = sbuf("der", [36, 8], F32)
    g_r = sbuf("g_r", [128, 4 * BL], F32)
    g_z = sbuf("g_z", [128, 4 * BL], F32)
    g_n = sbuf("g_n", [128, 4 * BL], F32)
    tmp_h = sbuf("tmp_h", [128, 4 * BL], F32)
    tmp_z = sbuf("tmp_z", [128, BL], F32)
    outw = sbuf("outw", [128, 1056], F32)
    biasc = sbuf("biasc", [128, 3], F32)   # cols: 64.5, ln(127/31), ln(1/sqrt2)

    rbuf_v = rbuf.rearrange("m (h n i) -> m h n i", h=2, n=NG, i=BL)
    wblk_v = wblk.rearrange("p (s i) -> p s i", s=NG, i=BL)
    rT_v = rT_bf.rearrange("p (k i) -> p k i", k=16, i=BL)
    wT_v = wT_bf.rearrange("p (u i) -> p u i", u=8, i=BL)

    ctx = contextlib.ExitStack()
    pool = ctx.enter_context(tc.tile_pool(name="ps", bufs=7, space="PSUM"))
    spool = ctx.enter_context(tc.tile_pool(name="sbp", bufs=3))

    for name, shape, dt in DRAM_SPECS:
        nc.sync.dma_start(sb[name][:], dram[name][:])
    for z in (hdec, henc, hdec_bf, henc_bf, recon, p5c, ball, der):
        nc.vector.memset(z[:], 0.0)
    nc.vector.memset(biasc[:, 0:1], 64.5)
    nc.vector.memset(biasc[:, 1:2], LN_DELTA_SCALE)
    nc.vector.memset(biasc[:, 2:3], LN_SQRT_HALF)

    pconst = sb["pconst"]
    ident = sb["ident"]
    identf = sb["identf"]
    sela = sb["sela"]
    selb = sb["selb"]

    # ---------------- helpers ----------------

    def gemm(lhsT_t, n_kt, n_mt, rhs_fn, psum_ap, mts, extra=None):
        """For mt in mts: psum[:, j*BL:(j+1)*BL] = sum over K tiles
        (main group then extra groups) of lhsT(kt, mt).T @ rhs_fn(kt)."""
        for j, mt in enumerate(mts):
            groups = [(n_kt, lhsT_t, rhs_fn)]
            if extra:
                groups = groups + list(extra)
            total = sum(g[0] for g in groups)
            done = 0
            for g_nkt, g_t, g_rs in groups:
                for kt in range(g_nkt):
                    nc.tensor.matmul(
                        psum_ap[:, j * BL:(j + 1) * BL],
                        g_t[:, (kt * n_mt + mt) * 128:
                            (kt * n_mt + mt + 1) * 128],
                        g_rs(kt),
                        start=(done == 0), stop=(done == total - 1))
                    done += 1

    def hslice(h_bf):
        return lambda kt: h_bf[:, kt * BL:(kt + 1) * BL]

    def build_scalars(p5_ps, w_side):
        bias_ap = sb["bias_wa" if w_side else "bias_ra"]
        nc.scalar.activation(p5r[:], p5_ps[:], AF.Identity,
                             bias=bias_ap[:, 0:1])
        p5t_ps = pool.tile([33, 8], F32, tag="ps")
        nc.tensor.matmul(p5t_ps[:], p5r[:, :BL], identf[:], is_transpose=True)
        nc.scalar.copy(p5c[:33, :], p5t_ps[:])
        # der cols: 0 g_x, 1 g_y, 2 delta, 3 rsts, 4 gamma(/1/gw), 5 -rsts
        nc.scalar.activation(der[:33, 0:1], p5c[:33, 0:1], AF.Identity,
                             bias=biasc[:33, 0:1], scale=64.5)
        nc.scalar.activation(der[:33, 1:2], p5c[:33, 1:2], AF.Identity,
                             bias=biasc[:33, 0:1], scale=64.5)
        nc.scalar.activation(der[:33, 2:3], p5c[:33, 3:4], AF.Exp,
                             bias=biasc[:33, 1:2])
        nc.scalar.activation(der[:33, 3:4], p5c[:33, 2:3], AF.Exp,
                             bias=biasc[:33, 2:3], scale=-0.5)
        nc.scalar.activation(der[:33, 4:5], p5c[:33, 4:5], AF.Exp,
                             scale=(-1.0 if w_side else 1.0))
        nc.vector.tensor_scalar(der[:33, 5:6], der[:33, 3:4], -1.0, None,
                                ALU.mult)
        ts = nc.vector.tensor_scalar
        ts(ball[0:36, 0:NT], selb[:], der[:36, 0:1], None, ALU.mult)
        ts(ball[64:100, 0:NT], selb[:], der[:36, 2:3], None, ALU.mult)
        ts(ball[0:36, NT:2 * NT], selb[:], der[:36, 1:2], None, ALU.mult)
        ts(ball[64:100, NT:2 * NT], selb[:], der[:36, 2:3], None, ALU.mult)
        ts(ball[0:36, 2 * NT:3 * NT], selb[:], der[:36, 3:4], None, ALU.mult)
        ts(ball[0:36, 3 * NT:4 * NT], selb[:], der[:36, 5:6], None, ALU.mult)
        ts(ball[0:36, 4 * NT:5 * NT], selb[:], der[:36, 4:5], None, ALU.mult)
        scal_ps = pool.tile([128, 5 * NT], F32, tag="ps")
        nc.tensor.matmul(scal_ps[:], sela[:], ball[:], start=True, stop=True)
        dst = scal_w if w_side else scal_r
        nc.scalar.copy(dst[:], scal_ps[:])
        bxd = bx_w if w_side else bx_r
        byd = by_w if w_side else by_r
        nc.vector.tensor_tensor(bxd[:], dst[:, 0:NT], dst[:, 3 * NT:4 * NT],
                                ALU.mult)
        nc.vector.tensor_tensor(byd[:], dst[:, NT:2 * NT],
                                dst[:, 3 * NT:4 * NT], ALU.mult)

    def build_banks(w_side):
        scal = scal_w if w_side else scal_r
        bxd = bx_w if w_side else bx_r
        byd = by_w if w_side else by_r
        bank_x = fxw_n if w_side else fx_n
        bank_y = fyw_n if w_side else fy_n
        off = 2 * NT if w_side else 0
        for gws, bias_t, sc in ((g_x, bxd, 0), (g_y, byd, 1)):
            for t in range(NT):
                nc.scalar.activation(
                    gws[:, t * 128:(t + 1) * 128], pconst[:],
                    AF.Square, bias=bias_t[:, t:t + 1],
                    scale=scal[:, 2 * NT + t:2 * NT + t + 1])
                nc.scalar.activation(
                    gws[:, t * 128:(t + 1) * 128],
                    gws[:, t * 128:(t + 1) * 128], AF.Exp, scale=-1.0)
                nc.vector.reduce_sum(
                    sums[:, off + sc * NT + t:off + sc * NT + t + 1],
                    gws[:, t * 128:(t + 1) * 128], axis=mybir.AxisListType.X)
        nc.vector.tensor_scalar(rcps[:, off:off + 2 * NT],
                                sums[:, off:off + 2 * NT], EPS_ADJ, None,
                                ALU.max)
        nc.vector.reciprocal(rcps[:, off:off + 2 * NT],
                             rcps[:, off:off + 2 * NT])
        nc.vector.tensor_tensor(rcps[:, off + NT:off + 2 * NT],
                                rcps[:, off + NT:off + 2 * NT],
                                scal[:, 4 * NT:5 * NT], ALU.mult)
        for gws, bank, sc in ((g_x, bank_x, 0), (g_y, bank_y, 1)):
            for t in range(NT):
                nc.scalar.activation(
                    bank[:, t * 128:(t + 1) * 128],
                    gws[:, t * 128:(t + 1) * 128], AF.Copy,
                    scale=rcps[:, off + sc * NT + t:off + sc * NT + t + 1])
        if not w_side:
            for t in range(NT):
                nc.sync.dma_start(fxT[:, t * 128:(t + 1) * 128],
                                  fx_n[:, t * 128:(t + 1) * 128],
                                  transpose=True)
                nc.sync.dma_start(fyT[:, t * 128:(t + 1) * 128],
                                  fy_n[:, t * 128:(t + 1) * 128],
                                  transpose=True)

    def gru(ps_rz, ps_in, ps_hn, h, h_bf, b_rz, b_in, b_hn):
        for m in range(4):
            nc.scalar.activation(g_r[:, m * BL:(m + 1) * BL],
                                 ps_rz[:, m * BL:(m + 1) * BL], AF.Sigmoid,
                                 bias=b_rz[:, m:m + 1])
            nc.scalar.activation(g_z[:, m * BL:(m + 1) * BL],
                                 ps_rz[:, (4 + m) * BL:(5 + m) * BL],
                                 AF.Sigmoid, bias=b_rz[:, 4 + m:5 + m])
            nc.vector.tensor_scalar(g_n[:, m * BL:(m + 1) * BL],
                                    ps_hn[:, m * BL:(m + 1) * BL],
                                    b_hn[:, m:m + 1], None, ALU.add)
        nc.vector.tensor_tensor(g_n[:], g_n[:], g_r[:], ALU.mult)
        nc.vector.tensor_tensor(g_n[:], g_n[:], ps_in[:], ALU.add)
        for m in range(4):
            nc.scalar.activation(g_n[:, m * BL:(m + 1) * BL],
                                 g_n[:, m * BL:(m + 1) * BL], AF.Tanh,
                                 bias=b_in[:, m:m + 1])
        nc.vector.tensor_tensor(tmp_h[:], h[:], g_n[:], ALU.subtract)
        nc.vector.tensor_tensor(tmp_h[:], tmp_h[:], g_z[:], ALU.mult)
        nc.vector.tensor_tensor(h[:], g_n[:], tmp_h[:], ALU.add)
        nc.vector.tensor_copy(h_bf[:], h[:])

    # ---------------- time steps ----------------
    for t in range(t_steps):
        # S = sigmoid(recon[replica]); S^T (stationary for the bulk matmul)
        nc.scalar.activation(sd_bf[:], recon[:, 32 * B_:33 * B_], AF.Sigmoid)
        sps = pool.tile([128, 128], BF16, tag="ps")
        nc.tensor.matmul(sps[:], sd_bf[:], ident[:], is_transpose=True)
        nc.scalar.copy(st_bf[:], sps[:])

        # read-side params + banks
        p5_ps = pool.tile([8, BL], F32, tag="ps")
        for kt in range(4):
            nc.tensor.matmul(p5_ps[:], sb["w_raT"][:, kt * 8:(kt + 1) * 8],
                             hdec[:, kt * BL:(kt + 1) * BL],
                             start=(kt == 0), stop=(kt == 3))
        build_scalars(p5_ps, w_side=False)
        build_banks(w_side=False)

        # bulk: szneg = -(S @ Fx^T) for all rows
        for c0 in range(0, NT * 128, 512):
            cw = min(512, NT * 128 - c0)
            sz_ps = pool.tile([128, 512], F32, tag="ps")
            nc.tensor.matmul(sz_ps[:, :cw], st_bf[:], fxT[:, c0:c0 + cw],
                             start=True, stop=True)
            nc.scalar.activation(szneg[:, c0:c0 + cw], sz_ps[:, :cw],
                                 AF.Copy, scale=-1.0)

        # per-row read attention
        for i in range(BL):
            tt, i4 = i // 4, i % 4
            col = tt * 128 + i4 * 32
            fx_sl = fxT[:, col:col + 32]
            fy_sl = fyT[:, col:col + 32]
            z_ps = pool.tile([128, 32], F32, tag="ps")
            nc.tensor.matmul(z_ps[:], sb["xT"][:, i * 128:(i + 1) * 128],
                             fx_sl, start=True, stop=True)
            nc.scalar.copy(zsb[:, i * 32:(i + 1) * 32], z_ps[:])
            xa_ps = pool.tile([32, 32], F32, tag="ps")
            nc.tensor.matmul(xa_ps[:], zsb[:, i * 32:(i + 1) * 32], fy_sl,
                             start=True, stop=True)
            nc.vector.tensor_copy(rbuf_v[:, 0, :, i], xa_ps[:])
            xh_ps = pool.tile([32, 32], F32, tag="ps")
            nc.tensor.matmul(xh_ps[:], zsb[:, i * 32:(i + 1) * 32], fy_sl,
                             start=True, stop=False)
            nc.tensor.matmul(xh_ps[:], szneg[:, col:col + 32], fy_sl,
                             start=False, stop=True)
            nc.vector.tensor_copy(rbuf_v[:, 1, :, i], xh_ps[:])

        # assemble r^T K-tiles (DMA per (half, q): out [128, 33] <- in
        # [32(m), 4(n4), 33(i)]; out partition p = m*4 + n4)
        for h in range(2):
            for q in range(8):
                nc.sync.dma_start(
                    rT_v[:, h * 8 + q, :],
                    rbuf_v[:, h, q * 4:(q + 1) * 4, :])

        # encoder GRU
        def enc_ih_rhs(kt):
            if kt < 16:
                return rT_bf[:, kt * BL:(kt + 1) * BL]
            return hdec_bf[:, (kt - 16) * BL:(kt - 16 + 1) * BL]

        ps_rz = pool.tile([128, 8 * BL], F32, tag="ps")
        ps_in = pool.tile([128, 4 * BL], F32, tag="ps")
        ps_hn = pool.tile([128, 4 * BL], F32, tag="ps")
        gemm(sb["w_ihT_enc"], 20, 12, enc_ih_rhs, ps_rz, range(8),
             extra=[(4, sb["w_hhT_enc"], hslice(henc_bf))])
        gemm(sb["w_ihT_enc"], 20, 12, enc_ih_rhs, ps_in, range(8, 12))
        gemm(sb["w_hhT_enc"], 4, 12, hslice(henc_bf), ps_hn, range(8, 12))
        gru(ps_rz, ps_in, ps_hn, henc, henc_bf, sb["bias_enc_rz"],
            sb["bias_enc_in"], sb["bias_enc_hn"])

        # z sample
        ps_muls = pool.tile([128, 2 * BL], F32, tag="ps")
        gemm(sb["w_mulsT"], 4, 2, hslice(henc), ps_muls, range(2))
        nc.scalar.activation(tmp_z[:], ps_muls[:, BL:2 * BL], AF.Exp,
                             bias=sb["bias_muls"][:, 1:2])
        nc.vector.tensor_tensor(tmp_z[:], tmp_z[:],
                                sb["epsT"][:, t * BL:(t + 1) * BL], ALU.mult)
        nc.vector.tensor_tensor(tmp_z[:], tmp_z[:], ps_muls[:, 0:BL], ALU.add)
        nc.vector.tensor_scalar(tmp_z[:], tmp_z[:], sb["bias_muls"][:, 0:1],
                                None, ALU.add)
        nc.vector.tensor_copy(zT_bf[:], tmp_z[:])

        # decoder GRU
        ps_rz2 = pool.tile([128, 8 * BL], F32, tag="ps")
        ps_in2 = pool.tile([128, 4 * BL], F32, tag="ps")
        ps_hn2 = pool.tile([128, 4 * BL], F32, tag="ps")
        zrhs = lambda kt: zT_bf[:]  # noqa: E731
        gemm(sb["w_ihT_dec"], 1, 12, zrhs, ps_rz2, range(8),
             extra=[(4, sb["w_hhT_dec"], hslice(hdec_bf))])
        gemm(sb["w_ihT_dec"], 1, 12, zrhs, ps_in2, range(8, 12))
        gemm(sb["w_hhT_dec"], 4, 12, hslice(hdec_bf), ps_hn2, range(8, 12))
        gru(ps_rz2, ps_in2, ps_hn2, hdec, hdec_bf, sb["bias_dec_rz"],
            sb["bias_dec_in"], sb["bias_dec_hn"])

        # write-side params + banks
        p5w_ps = pool.tile([8, BL], F32, tag="ps")
        for kt in range(4):
            nc.tensor.matmul(p5w_ps[:], sb["w_waT"][:, kt * 8:(kt + 1) * 8],
                             hdec[:, kt * BL:(kt + 1) * BL],
                             start=(kt == 0), stop=(kt == 3))
        build_scalars(p5w_ps, w_side=True)
        build_banks(w_side=True)

        # w gemm
        ps_w = pool.tile([128, 8 * BL], F32, tag="ps")
        gemm(sb["w_wrT"], 4, 8, hslice(hdec_bf), ps_w, range(8))
        for u in range(8):
            nc.scalar.activation(wT_bf[:, u * BL:(u + 1) * BL],
                                 ps_w[:, u * BL:(u + 1) * BL], AF.Identity,
                                 bias=sb["bias_wr"][:, u:u + 1])

        # wblk: [n, (s, i)] at base 0 (s = u*4 + s4; in partition r = n*4+s4),
        # then replicate to bases 32/64/96
        # per M-tile u: in = wT[:, u, :] [128(r=(n,j)), 33]; out [32(n), 4(j), 33]
        wblk0_v = wblk[0:32, :].rearrange("n (u j i) -> n u j i",
                                          u=8, j=4, i=BL)
        for u in range(8):
            nc.sync.dma_start(wblk0_v[:, u, :, :], wT_v[:, u, :])
        for b in (32, 64, 96):
            nc.sync.dma_start(wblk[b:b + 32, :], wblk[0:32, :])

        # per-row write attention
        for g in range(NT):
            rows = [4 * g + i4 for i4 in range(4) if 4 * g + i4 < BL]
            pa_ps = pool.tile([128, 128], F32, tag="ps")
            for i in rows:
                i4 = i % 4
                nc.tensor.matmul(
                    pa_ps[i4 * 32:(i4 + 1) * 32, :],
                    wblk_v[i4 * 32:(i4 + 1) * 32, :, i],
                    fyw_n[i4 * 32:(i4 + 1) * 32, g * 128:(g + 1) * 128],
                    start=True, stop=True,
                    tile_position=(32 * i4, 32 * i4))
            pt_sb = spool.tile([128, 128], BF16, tag="ptsb")
            nr = 32 * len(rows)
            nc.scalar.copy(pt_sb[:nr, :], pa_ps[:nr, :])
            for i in rows:
                i4 = i % 4
                pb_ps = pool.tile([128, 128], F32, tag="ps")
                nc.tensor.matmul(
                    pb_ps[:], pt_sb[i4 * 32:(i4 + 1) * 32, :],
                    fxw_n[i4 * 32:(i4 + 1) * 32, g * 128:(g + 1) * 128],
                    start=True, stop=True, tile_position=(32 * i4, 0))
                nc.vector.tensor_tensor(recon[:, i * B_:(i + 1) * B_],
                                        recon[:, i * B_:(i + 1) * B_],
                                        pb_ps[:], ALU.add)

        if history:
            hr = nc.dram_tensor(f"hist_recon_{t}", [128, BL * B_], F32,
                                kind="ExternalOutput").ap()
            nc.sync.dma_start(hr[:], recon[:])
            hh = nc.dram_tensor(f"hist_h_{t}", [128, 8 * BL], F32,
                                kind="ExternalOutput").ap()
            nc.sync.dma_start(hh[:, 0:4 * BL], henc[:])
            nc.sync.dma_start(hh[:, 4 * BL:8 * BL], hdec[:])

    # debug dumps
    for dn in debug_names:
        ap = sb[dn]
        dbg = nc.dram_tensor("dbg_" + dn, list(ap.shape), ap.dtype,
                             kind="ExternalOutput").ap()
        nc.sync.dma_start(dbg[:], ap[:])

    # output: sigmoid(recon) -> DRAM
    for c0 in range(0, BL * B_, 1056):
        cw = min(1056, BL * B_ - c0)
        nc.scalar.activation(outw[:, :cw], recon[:, c0:c0 + cw], AF.Sigmoid)
        nc.sync.dma_start(out_d[:, c0:c0 + cw], outw[:, :cw])

    ctx.close()


# ----------------------------------------------------------------------------
# entry
# ----------------------------------------------------------------------------

_CACHE = {}


def _get_program(t_steps=T):
    if t_steps not in _CACHE:
        _CACHE[t_steps] = build_program(t_steps)
    return _CACHE[t_steps]


def gather_output(results):
    outs = []
    for c in range(NCORES):
        o = np.asarray(results[c]["out_sig"])
        o = o.reshape(128, BL, B_).transpose(1, 0, 2)
        outs.append(o[:32])
    return np.concatenate(outs, 0).reshape(BATCH, 1, A_, B_).astype(np.float32)


def kernel(**inputs):
    nc = _get_program(T)
    per_core = prepare_inputs(inputs)
    res = run_bass_kernel_spmd(nc, per_core, list(range(NCORES)))
    return gather_output(res.results)

